# revision 60
# baseline (speedup 1.0000x reference)
"""Trainium2 Bass kernel for nn_Attention_65747359367242.

Per-batch tanh-attention with head-mean:
  Q = x@Wq+bq, K = cond@Wk+bk, V = cond@Wv+bv   (4 heads of 32 dims)
  S_h = Q_h K_h^T / sqrt(128)
  A   = mean_h tanh(mask + S_h)
  out = A @ V

Sharding: pure data-parallel, batch b -> core b (B=8, 8 cores). No collectives.

Device strategy per core (transposed orientation: scores S^T[m, n]):
  - host feeds x^T, cond^T, mask^T (bf16) + prescaled weights
  - Q^T/K^T/V computed on device via small matmuls (biases added as rank-1
    K=1 matmuls accumulating into the same PSUM)
  - main loop over (n-chunk 512, m-tile 128), half-group (head-pair)
    pipelined so each PSUM tile flows PE -> ACT -> DVE independently:
      * pair 0 (heads 0,1): mask^T injected into both banks by full-array
        identity matmuls (start=True -> bank clear + barrier), then 8
        tile-position-packed K=32 score matmuls accumulate S_h^T on top
      * pair 1 (heads 2,3): each bank's scores are ONE K=32/M=128 matmul
        (start=True, single writer -> race-free), then VectorE adds
        mask^T into PSUM in place (balances PE vs DVE under the power
        duty-cycle throttle)
      * ScalarE tanh PSUM -> SBUF bf16 per head-pair
      * VectorE adds each pair's heads (scale 1/4 folded into Wv/bv)
      * one AV matmul per m-tile, emitted one group LATE so it reaches
        the in-order PE queue with its input ready
  - out^T streamed to DRAM; host transposes back.
"""

import math
import sys

import numpy as np

sys.path.insert(0, "/opt/trn_rl_repo")

B, N, D = 8, 2048, 128
H, DH = 4, 32
NCH = 512            # n-chunk (free dim of score tiles / psum bank)
N_NCH = N // NCH     # 4
N_MT = N // 128      # 16 m-tiles

_NC_CACHE = {}


def _build_nc():
    from concourse import bass, tile
    from concourse.tile import add_dep_helper

    mybir = sys.modules["concourse.mybir"]
    f32 = mybir.dt.float32
    bf16 = mybir.dt.bfloat16
    TANH = mybir.ActivationFunctionType.Tanh

    nc = bass.Bass()

    xT = nc.declare_dram_parameter("xT", [D, N], bf16, isOutput=False)
    condT = nc.declare_dram_parameter("condT", [D, N], bf16, isOutput=False)
    maskT = nc.declare_dram_parameter("maskT", [N, N], bf16, isOutput=False)
    Wq = nc.declare_dram_parameter("Wq", [D, D], bf16, isOutput=False)
    Wk = nc.declare_dram_parameter("Wk", [D, D], bf16, isOutput=False)
    Wv4 = nc.declare_dram_parameter("Wv4", [D, D], bf16, isOutput=False)
    bq = nc.declare_dram_parameter("bq", [D, D], bf16, isOutput=False)
    bk = nc.declare_dram_parameter("bk", [D, D], bf16, isOutput=False)
    bv4 = nc.declare_dram_parameter("bv4", [D, D], bf16, isOutput=False)
    onesm = nc.declare_dram_parameter("onesm", [D, NCH], bf16, isOutput=False)
    eyef = nc.declare_dram_parameter("eyef", [D, D], bf16, isOutput=False)
    outT = [nc.declare_dram_parameter(f"outT{i}", [D, NCH], f32,
                                      isOutput=True) for i in range(N_NCH)]

    with tile.TileContext(nc) as tc:
        with (
            tc.tile_pool(name="const", bufs=1) as cpool,
            tc.tile_pool(name="proj", bufs=1) as projpool,
            tc.tile_pool(name="mask", bufs=64) as mpool,
            tc.tile_pool(name="th", bufs=10) as thpool,
            tc.tile_pool(name="at", bufs=3) as atpool,
            tc.tile_pool(name="osb", bufs=4) as opool,
            tc.tile_pool(name="ps", bufs=3, space="PSUM") as pspool,
            tc.tile_pool(name="av", bufs=2, space="PSUM") as avpool,
            tc.tile_pool(name="gsb", bufs=70) as gsbpool,
        ):
            # ---- load constants / inputs ----
            wq_sb = cpool.tile([D, D], bf16, tag="wq")
            wk_sb = cpool.tile([D, D], bf16, tag="wk")
            wv_sb = cpool.tile([D, D], bf16, tag="wv")
            bq_sb = cpool.tile([D, D], bf16, tag="bq")
            bk_sb = cpool.tile([D, D], bf16, tag="bk")
            bv_sb = cpool.tile([D, D], bf16, tag="bv")
            ones_sb = cpool.tile([D, NCH], bf16, tag="ones")
            eyef_sb = cpool.tile([D, D], bf16, tag="eyef")
            xT_sb = cpool.tile([D, N], bf16, tag="xT")
            condT_sb = cpool.tile([D, N], bf16, tag="condT")

            # ldweights gates absorb DMA waits on the PE side (the Matmult
            # HW struct fits only one sync wait). They must be FULL-HEIGHT
            # [128, 1] loads: partial-height standalone ldweights before
            # tile_position matmuls hard-fault the PE
            # (NRT_EXEC_UNIT_UNRECOVERABLE).
            for sb_t, dr_t in [(wq_sb, Wq), (wk_sb, Wk), (wv_sb, Wv4),
                               (eyef_sb, eyef),
                               (xT_sb, xT), (condT_sb, condT)]:
                nc.sync.dma_start(out=sb_t[:], in_=dr_t[:])
                nc.tensor.ldweights(sb_t[:, 0:1])
            for sb_t, dr_t in [(bq_sb, bq), (bk_sb, bk), (bv_sb, bv4),
                               (ones_sb, onesm)]:
                nc.sync.dma_start(out=sb_t[:], in_=dr_t[:])
                nc.tensor.ldweights(sb_t[:, 0:1])

            # ---- projections ----
            # Q^T[d, n] = Wq'^T x^T + bq' x ones ; same for K^T. V[m, d] chunks.
            qT_sb = projpool.tile([D, N], bf16, tag="qT")
            kT_sb = projpool.tile([D, N], bf16, tag="kT")
            v_sb = projpool.tile([128, N], bf16, tag="v")  # chunk m at free 128m

            for c in range(N_NCH):
                sl = slice(c * NCH, (c + 1) * NCH)
                pq = pspool.tile([128, 2 * NCH], f32, tag="sc")
                nc.tensor.matmul(pq[:, 0:NCH], wq_sb[:], xT_sb[:, sl],
                                 start=True, stop=False)
                nc.tensor.matmul(pq[:, 0:NCH], bq_sb[:], ones_sb[:],
                                 start=False, stop=True)
                nc.vector.tensor_copy(qT_sb[:, sl], pq[:, 0:NCH])

                pk = pspool.tile([128, 2 * NCH], f32, tag="sc")
                nc.tensor.matmul(pk[:, 0:NCH], wk_sb[:], condT_sb[:, sl],
                                 start=True, stop=False)
                nc.tensor.matmul(pk[:, 0:NCH], bk_sb[:], ones_sb[:],
                                 start=False, stop=True)
                nc.vector.tensor_copy(kT_sb[:, sl], pk[:, 0:NCH])

            for t in range(N_MT):
                sl = slice(t * 128, (t + 1) * 128)
                pv = pspool.tile([128, 2 * NCH], f32, tag="sc")
                nc.tensor.matmul(pv[:, 0:D], condT_sb[:, sl], wv_sb[:],
                                 start=True, stop=False)
                nc.tensor.matmul(pv[:, 0:D], ones_sb[:, 0:128], bv_sb[:],
                                 start=False, stop=True)  # row0-padded rank-1
                last_vcopy = nc.vector.tensor_copy(v_sb[:, sl], pv[:, 0:D])

            # small ACT-written source tile for the gact2 gates
            actsrc = cpool.tile([1, 8], bf16, tag="actsrc")
            nc.scalar.copy(actsrc[0:1, 0:1], qT_sb[0:1, 0:1])
            # one-time DVE gate: advance DVE's observed self clock past the
            # projection copies so group 0's in-place mask adds elide them
            g0t = cpool.tile([1, 8], bf16, tag="g0t")
            g0 = nc.vector.tensor_copy(g0t[0:1, 0:1], qT_sb[0:1, 0:1])
            add_dep_helper(g0.ins, last_vcopy.ins, reason="proj fence")

            # ---- main loop ----
            # The whole bf16 mask^T fits in SBUF (64 KiB/partition): issue
            # all 64 tile DMAs up-front into fresh slots. Fresh tiles carry
            # no WAR/WAW waits, which keeps every DMA within the sync-wait
            # slot budget, and gives maximal prefetch depth.
            mk_tiles = {}
            tail_insts = []
            for ncg in range(N_NCH):
                for mt in range(N_MT):
                    mk = mpool.tile([128, NCH], bf16, tag="mk",
                                    name=f"mk_{ncg}_{mt}", bufs=64)
                    dmi = nc.sync.dma_start(
                        out=mk[:],
                        in_=maskT[mt * 128:(mt + 1) * 128,
                                  ncg * NCH:(ncg + 1) * NCH],
                    )
                    if ncg == N_NCH - 1 and mt >= N_MT - 8:
                        tail_insts.append(dmi)
                    mk_tiles[(ncg, mt)] = mk

            # Half-group pipeline: each PSUM tile (one head-PAIR x one m-tile
            # x one n-chunk) flows through PE -> ACT -> DVE independently, at
            # half-group granularity. The inject of half k is gated on
            # tanh(k-3) (its PSUM slot release, bufs=3), so the PE never
            # waits a whole group for ACT, stays dense, and HAM stays warm.
            GATE_EVERY = 6  # spacing of the clock-advance gates (halves)
            tanh_hist = []   # all tanh insts, by half index
            madds_hist = []  # per half: the DVE mask-add insts (p1 only)
            prev_at = None
            prev_av_mm = None
            last_av_mm = None
            half = 0
            # AV matmuls are emitted one group LATE so they reach the head
            # of the in-order PE queue with their DVE input (at) already
            # computed -- otherwise each AV head-of-line-blocks the PE for
            # ~3us per group and the pipeline never tightens.
            delayed_av = []  # (av_tile, mt, at_tile, start, stop, ncg)

            def flush_delayed():
                nonlocal prev_av_mm, last_av_mm
                for av_t, fmt, at_t, st, sp, fncg in delayed_av:
                    mmav = nc.tensor.matmul(
                        av_t[:], v_sb[:, fmt * 128:(fmt + 1) * 128], at_t[:],
                        start=st, stop=sp,
                    )
                    prev_av_mm = mmav
                    last_av_mm = mmav
                    if sp:
                        oT = opool.tile([D, NCH], f32, tag="oT")
                        cp = nc.vector.tensor_copy(oT[:], av_t[:])
                        od = nc.gpsimd.dma_start(out=outT[fncg][:], in_=oT[:])
                        tail_insts.extend([cp, od])
                delayed_av.clear()

            for ncg in range(N_NCH):
                nsl = slice(ncg * NCH, (ncg + 1) * NCH)
                av = avpool.tile([D, NCH], f32, tag="av")
                for mt in range(N_MT):
                    mk = mk_tiles[(ncg, mt)]
                    # PE gate: absorbs the mask DMA wait.
                    nc.tensor.ldweights(mk[:, 0:1])
                    uw = []
                    for p in range(2):
                        # PE gate: absorbs the PSUM-slot-release (ACT) wait.
                        g2 = nc.tensor.ldweights(eyef_sb[:, 0:1])
                        if len(tanh_hist) >= 3:
                            add_dep_helper(g2.ins, tanh_hist[-3].ins,
                                           reason="sc release")
                        if len(madds_hist) >= 3 and madds_hist[-3]:
                            # second PE gate: absorbs the DVE (mask-add)
                            # part of the reused slot's release.
                            g3 = nc.tensor.ldweights(eyef_sb[:, 0:1])
                            for ma in madds_hist[-3]:
                                add_dep_helper(g3.ins, ma.ins,
                                               reason="sc release dve")
                        sch = pspool.tile([128, 2 * NCH], f32, tag="sc",
                                          name=f"sc{p}")
                        madds = []
                        if p == 0:
                            # inject mask^T into each head's bank with a
                            # FULL-ARRAY identity matmul (tiled injects
                            # racing the tiled score accumulation on the
                            # same PSUM addresses hard-fault the PE).
                            for hh in range(2):
                                off = NCH * hh
                                mm = nc.tensor.matmul(
                                    sch[:, off:off + NCH], eyef_sb[:], mk[:],
                                    start=True, stop=False,
                                    skip_group_check=True,
                                )
                                add_dep_helper(mm.ins, g2.ins,
                                               reason="gate order")
                            # packed scores for this pair's two heads
                            for j in range(4):
                                qs = slice(32 * j, 32 * (j + 1))
                                moff = mt * 128 + 32 * j
                                for hh in range(2):
                                    h = 2 * p + hh
                                    hs = slice(32 * h, 32 * (h + 1))
                                    off = NCH * hh
                                    nc.tensor.matmul(
                                        sch[qs, off:off + NCH],
                                        kT_sb[hs, moff:moff + 32],
                                        qT_sb[hs, nsl],
                                        start=False, stop=(j == 3),
                                        tile_position=(32 * h, 32 * j),
                                        skip_group_check=True,
                                    )
                        else:
                            # VectorE carries this pair's mask add: each
                            # bank's scores are ONE K=32/M=128 matmul with
                            # start=True (single writer per bank -- no
                            # packed clears to race), then DVE adds mask^T
                            # into PSUM in place.
                            score_stop = []
                            for hh in range(2):
                                h = 2 * p + hh
                                hs = slice(32 * h, 32 * (h + 1))
                                off = NCH * hh
                                smm = nc.tensor.matmul(
                                    sch[:, off:off + NCH],
                                    kT_sb[hs, mt * 128:(mt + 1) * 128],
                                    qT_sb[hs, nsl],
                                    start=True, stop=True,
                                    tile_position=(32 * h, 0),
                                    skip_group_check=True,
                                )
                                add_dep_helper(smm.ins, g2.ins,
                                               reason="gate order")
                                score_stop.append(smm)
                            # DVE gate: absorbs the mask-DMA wait so the
                            # in-place adds carry only their PE wait.
                            gdmk = gsbpool.tile([1, 8], bf16, tag="gdmk",
                                                name=f"gdmk_{half}", bufs=70)
                            nc.vector.tensor_copy(gdmk[0:1, 0:1],
                                                  mk[0:1, 0:1])
                            for hh in range(2):
                                off = NCH * hh
                                ma = nc.vector.tensor_add(
                                    sch[:, off:off + NCH],
                                    sch[:, off:off + NCH], mk[:])
                                madds.append(ma)
                        madds_hist.append(madds)
                        # spaced ACT gates: advance ACT's observed DVE clock
                        # (th slot releases) and ACT self clock (th WAW) so
                        # the real tanhs carry only their PE wait.
                        if half % GATE_EVERY == 2 and prev_at is not None:
                            gact = gsbpool.tile([1, 8], bf16, tag="gact",
                                                name=f"gact_{half}", bufs=70)
                            nc.scalar.copy(gact[0:1, 0:1], prev_at[0:1, 0:1])
                            gact2 = gsbpool.tile([1, 8], bf16, tag="gact2",
                                                 name=f"gact2_{half}", bufs=70)
                            ga2 = nc.scalar.copy(gact2[0:1, 0:1],
                                                 actsrc[0:1, 0:1])
                            add_dep_helper(ga2.ins, tanh_hist[-1].ins,
                                           reason="th waw")
                        if p == 1:
                            # ACT gate: absorbs the PE score wait so this
                            # tanh carries only the DVE mask-add wait.
                            gactp = gsbpool.tile([1, 8], bf16, tag="gactp",
                                                 name=f"gactp_{half}",
                                                 bufs=70)
                            gp = nc.scalar.copy(gactp[0:1, 0:1],
                                                qT_sb[0:1, 0:1])
                            for smm in score_stop:
                                add_dep_helper(gp.ins, smm.ins,
                                               reason="score order")
                        tht = thpool.tile([128, 2 * NCH], bf16, tag="th",
                                          name=f"th{p}")
                        act = nc.scalar.activation(tht[:], sch[:], TANH)
                        tanh_hist.append(act)
                        # pair-sum on DVE (bf16 SBUF 2x mode)
                        uwt = atpool.tile([128, NCH], bf16,
                                          tag=("u" if p == 0 else "w"))
                        nc.vector.tensor_add(uwt[:], tht[:, 0:NCH],
                                             tht[:, NCH:2 * NCH])
                        uw.append(uwt)
                        half += 1
                    # flush the PREVIOUS group's AV matmul only now, at the
                    # end of this group's PE stream: it reaches the in-order
                    # PE queue with its DVE input long finished, instead of
                    # splitting this group's score pack with a ~1us wait.
                    flush_delayed()
                    # DVE gate: absorbs the at-slot-release (PE) wait so the
                    # adds carry at most one sync wait.
                    if prev_av_mm is not None and mt % 2 == 0:
                        gdve = gsbpool.tile([1, 8], bf16, tag="gdve",
                                            name=f"gdve_{ncg}_{mt}", bufs=70)
                        gd = nc.vector.tensor_copy(gdve[0:1, 0:1],
                                                   actsrc[0:1, 0:1])
                        add_dep_helper(gd.ins, prev_av_mm.ins,
                                       reason="at release")
                    at = atpool.tile([128, NCH], bf16, tag="at")
                    nc.vector.tensor_add(at[:], uw[0][:], uw[1][:])
                    prev_at = at
                    # out^T[d, nch] += V'[mt]^T @ A^T[mt] (deferred)
                    delayed_av.append((av, mt, at,
                                       mt == 0, mt == N_MT - 1, ncg))
            flush_delayed()

            tail_insts.extend(tanh_hist[-2:])
            if last_av_mm is not None:
                tail_insts.append(last_av_mm)
            for ti in tail_insts:
                nz = nc.sync.nop(nofuse=True, hint="predrain")
                add_dep_helper(nz.ins, ti.ins, reason="predrain absorb")

    return nc


def get_nc():
    if "nc" not in _NC_CACHE:
        _NC_CACHE["nc"] = _build_nc()
    return _NC_CACHE["nc"]


def _prep_in_maps(x, cond, attention_mask, Wq, bq, Wk, bk, Wv, bv):
    import ml_dtypes

    bf16 = ml_dtypes.bfloat16
    s = 1.0 / math.sqrt(128.0)

    Wq_s = (np.asarray(Wq, np.float32) * s).astype(bf16)
    Wk_b = np.asarray(Wk, np.float32).astype(bf16)
    Wv4 = (np.asarray(Wv, np.float32) * 0.25).astype(bf16)

    def _row0(vec):
        m = np.zeros((D, D), np.float32)
        m[0, :] = vec
        return m.astype(bf16)

    bq_s = _row0(np.asarray(bq, np.float32) * s)
    bk_b = _row0(np.asarray(bk, np.float32))
    bv4 = _row0(np.asarray(bv, np.float32) * 0.25)
    onesm = np.zeros((D, NCH), np.float32)
    onesm[0, :] = 1.0
    onesm = onesm.astype(bf16)
    eyef = np.eye(D, dtype=np.float32).astype(bf16)

    x = np.asarray(x, np.float32)
    cond = np.asarray(cond, np.float32)
    attention_mask = np.asarray(attention_mask, np.float32)

    in_maps = []
    for i in range(B):
        in_maps.append({
            "xT": np.ascontiguousarray(x[i].T).astype(bf16),
            "condT": np.ascontiguousarray(cond[i].T).astype(bf16),
            "maskT": np.ascontiguousarray(attention_mask[i].T).astype(bf16),
            "Wq": Wq_s, "Wk": Wk_b, "Wv4": Wv4,
            "bq": bq_s, "bk": bk_b, "bv4": bv4,
            "onesm": onesm, "eyef": eyef,
        })
    return in_maps


def run(x, cond, flags, attention_mask, Wq, bq, Wk, bk, Wv, bv,
        trace=False, tmpdir=None):
    """Returns (out [B,N,D] float32, exec_time_ns or None)."""
    from concourse.bass_utils import run_bass_kernel_spmd

    nc = get_nc()
    in_maps = _prep_in_maps(x, cond, attention_mask, Wq, bq, Wk, bk, Wv, bv)
    res = run_bass_kernel_spmd(
        nc, in_maps, core_ids=list(range(B)), trace=trace, tmpdir=tmpdir,
    )
    out = np.stack(
        [np.concatenate([np.asarray(r[f"outT{i}"], np.float32)
                         for i in range(N_NCH)], axis=1).T
         for r in res.results], axis=0
    )
    return out, res.exec_time_ns


def kernel(**inputs):
    out, _ = run(**inputs)
    return out


# revision 63
# speedup vs baseline: 1.1860x; 1.1860x over previous
"""Trainium2 Bass kernel for nn_Attention_65747359367242.

Per-batch tanh-attention with head-mean:
  Q = x@Wq+bq, K = cond@Wk+bk, V = cond@Wv+bv   (4 heads of 32 dims)
  S_h = Q_h K_h^T / sqrt(128)
  A   = mean_h tanh(mask + S_h)
  out = A @ V

Sharding: pure data-parallel, batch b -> core b (B=8, 8 cores). No collectives.

Device strategy per core (transposed orientation: scores S^T[m, n]):
  - host feeds x^T, cond^T, mask^T (bf16) + prescaled weights
  - Q^T/K^T/V computed on device via small matmuls (biases added as rank-1
    K=1 matmuls accumulating into the same PSUM)
  - main loop over (n-chunk 512, m-tile 128), half-group (head-pair)
    pipelined so each PSUM tile flows PE -> ACT -> DVE independently:
      * pair 0 (heads 0,1): mask^T injected into both banks by full-array
        identity matmuls (start=True -> bank clear + barrier), then 8
        tile-position-packed K=32 score matmuls accumulate S_h^T on top
      * pair 1 (heads 2,3): each bank's scores are ONE K=32/M=128 matmul
        (start=True, single writer -> race-free), then VectorE adds
        mask^T into PSUM in place (balances PE vs DVE under the power
        duty-cycle throttle)
      * ScalarE tanh PSUM -> SBUF bf16 per head-pair
      * VectorE adds each pair's heads (scale 1/4 folded into Wv/bv)
      * one AV matmul per m-tile, emitted one group LATE so it reaches
        the in-order PE queue with its input ready
  - out^T streamed to DRAM; host transposes back.
"""

import math
import sys

import numpy as np

sys.path.insert(0, "/opt/trn_rl_repo")

B, N, D = 8, 2048, 128
H, DH = 4, 32
NCH = 512            # n-chunk (free dim of score tiles / psum bank)
N_NCH = N // NCH     # 4
N_MT = N // 128      # 16 m-tiles

_NC_CACHE = {}


def _build_nc():
    from concourse import bass, tile
    from concourse.tile import add_dep_helper

    mybir = sys.modules["concourse.mybir"]
    f32 = mybir.dt.float32
    bf16 = mybir.dt.bfloat16
    TANH = mybir.ActivationFunctionType.Tanh

    nc = bass.Bass()

    xT = nc.declare_dram_parameter("xT", [D, N], bf16, isOutput=False)
    condT = nc.declare_dram_parameter("condT", [D, N], bf16, isOutput=False)
    maskT = nc.declare_dram_parameter("maskT", [N, N], bf16, isOutput=False)
    Wq = nc.declare_dram_parameter("Wq", [D, D], bf16, isOutput=False)
    Wk = nc.declare_dram_parameter("Wk", [D, D], bf16, isOutput=False)
    Wv4 = nc.declare_dram_parameter("Wv4", [D, D], bf16, isOutput=False)
    bq = nc.declare_dram_parameter("bq", [D, D], bf16, isOutput=False)
    bk = nc.declare_dram_parameter("bk", [D, D], bf16, isOutput=False)
    bv4 = nc.declare_dram_parameter("bv4", [D, D], bf16, isOutput=False)
    onesm = nc.declare_dram_parameter("onesm", [D, NCH], bf16, isOutput=False)
    eyef = nc.declare_dram_parameter("eyef", [D, D], bf16, isOutput=False)
    outT = [nc.declare_dram_parameter(f"outT{i}", [D, NCH], f32,
                                      isOutput=True) for i in range(N_NCH)]

    with tile.TileContext(nc) as tc:
        with (
            tc.tile_pool(name="const", bufs=1) as cpool,
            tc.tile_pool(name="proj", bufs=1) as projpool,
            tc.tile_pool(name="mask", bufs=64) as mpool,
            tc.tile_pool(name="th", bufs=10) as thpool,
            tc.tile_pool(name="at", bufs=3) as atpool,
            tc.tile_pool(name="osb", bufs=4) as opool,
            tc.tile_pool(name="ps", bufs=3, space="PSUM") as pspool,
            tc.tile_pool(name="av", bufs=2, space="PSUM") as avpool,
            tc.tile_pool(name="gsb", bufs=70) as gsbpool,
        ):
            # ---- load constants / inputs ----
            wq_sb = cpool.tile([D, D], bf16, tag="wq")
            wk_sb = cpool.tile([D, D], bf16, tag="wk")
            wv_sb = cpool.tile([D, D], bf16, tag="wv")
            bq_sb = cpool.tile([D, D], bf16, tag="bq")
            bk_sb = cpool.tile([D, D], bf16, tag="bk")
            bv_sb = cpool.tile([D, D], bf16, tag="bv")
            ones_sb = cpool.tile([D, NCH], bf16, tag="ones")
            eyef_sb = cpool.tile([D, D], bf16, tag="eyef")
            xT_sb = cpool.tile([D, N], bf16, tag="xT")
            condT_sb = cpool.tile([D, N], bf16, tag="condT")

            # ldweights gates absorb DMA waits on the PE side (the Matmult
            # HW struct fits only one sync wait). They must be FULL-HEIGHT
            # [128, 1] loads: partial-height standalone ldweights before
            # tile_position matmuls hard-fault the PE
            # (NRT_EXEC_UNIT_UNRECOVERABLE).
            for sb_t, dr_t in [(wq_sb, Wq), (wk_sb, Wk), (wv_sb, Wv4),
                               (eyef_sb, eyef),
                               (xT_sb, xT), (condT_sb, condT)]:
                nc.sync.dma_start(out=sb_t[:], in_=dr_t[:])
                nc.tensor.ldweights(sb_t[:, 0:1])
            for sb_t, dr_t in [(bq_sb, bq), (bk_sb, bk), (bv_sb, bv4),
                               (ones_sb, onesm)]:
                nc.sync.dma_start(out=sb_t[:], in_=dr_t[:])
                nc.tensor.ldweights(sb_t[:, 0:1])

            # ---- projections ----
            # Q^T[d, n] = Wq'^T x^T + bq' x ones ; same for K^T. V[m, d] chunks.
            qT_sb = projpool.tile([D, N], bf16, tag="qT")
            kT_sb = projpool.tile([D, N], bf16, tag="kT")
            v_sb = projpool.tile([128, N], bf16, tag="v")  # chunk m at free 128m

            for c in range(N_NCH):
                sl = slice(c * NCH, (c + 1) * NCH)
                pq = pspool.tile([128, 2 * NCH], f32, tag="sc")
                nc.tensor.matmul(pq[:, 0:NCH], wq_sb[:], xT_sb[:, sl],
                                 start=True, stop=False)
                nc.tensor.matmul(pq[:, 0:NCH], bq_sb[:], ones_sb[:],
                                 start=False, stop=True)
                nc.vector.tensor_copy(qT_sb[:, sl], pq[:, 0:NCH])

                pk = pspool.tile([128, 2 * NCH], f32, tag="sc")
                nc.tensor.matmul(pk[:, 0:NCH], wk_sb[:], condT_sb[:, sl],
                                 start=True, stop=False)
                nc.tensor.matmul(pk[:, 0:NCH], bk_sb[:], ones_sb[:],
                                 start=False, stop=True)
                nc.vector.tensor_copy(kT_sb[:, sl], pk[:, 0:NCH])

            for t in range(N_MT):
                sl = slice(t * 128, (t + 1) * 128)
                pv = pspool.tile([128, 2 * NCH], f32, tag="sc")
                nc.tensor.matmul(pv[:, 0:D], condT_sb[:, sl], wv_sb[:],
                                 start=True, stop=False)
                nc.tensor.matmul(pv[:, 0:D], ones_sb[:, 0:128], bv_sb[:],
                                 start=False, stop=True)  # row0-padded rank-1
                last_vcopy = nc.vector.tensor_copy(v_sb[:, sl], pv[:, 0:D])

            # small ACT-written source tile for the gact2 gates
            actsrc = cpool.tile([1, 8], bf16, tag="actsrc")
            nc.scalar.copy(actsrc[0:1, 0:1], qT_sb[0:1, 0:1])
            # one-time DVE gate: advance DVE's observed self clock past the
            # projection copies so group 0's in-place mask adds elide them
            g0t = cpool.tile([1, 8], bf16, tag="g0t")
            g0 = nc.vector.tensor_copy(g0t[0:1, 0:1], qT_sb[0:1, 0:1])
            add_dep_helper(g0.ins, last_vcopy.ins, reason="proj fence")

            # ---- main loop ----
            # The whole bf16 mask^T fits in SBUF (64 KiB/partition): issue
            # all 64 tile DMAs up-front into fresh slots. Fresh tiles carry
            # no WAR/WAW waits, which keeps every DMA within the sync-wait
            # slot budget, and gives maximal prefetch depth.
            mk_tiles = {}
            tail_insts = []
            for ncg in range(N_NCH):
                for mt in range(N_MT):
                    mk = mpool.tile([128, NCH], bf16, tag="mk",
                                    name=f"mk_{ncg}_{mt}", bufs=64)
                    dmi = nc.sync.dma_start(
                        out=mk[:],
                        in_=maskT[mt * 128:(mt + 1) * 128,
                                  ncg * NCH:(ncg + 1) * NCH],
                    )
                    if ncg == N_NCH - 1 and mt >= N_MT - 8:
                        tail_insts.append(dmi)
                    mk_tiles[(ncg, mt)] = mk

            # Half-group pipeline: each PSUM tile (one head-PAIR x one m-tile
            # x one n-chunk) flows through PE -> ACT -> DVE independently, at
            # half-group granularity. The inject of half k is gated on
            # tanh(k-3) (its PSUM slot release, bufs=3), so the PE never
            # waits a whole group for ACT, stays dense, and HAM stays warm.
            GATE_EVERY = 6  # spacing of the clock-advance gates (halves)
            tanh_hist = []   # all tanh insts, by half index
            madds_hist = []  # per half: the DVE mask-add insts (p1 only)
            prev_at = None
            prev_av_mm = None
            last_av_mm = None
            half = 0
            # AV matmuls are emitted one group LATE so they reach the head
            # of the in-order PE queue with their DVE input (at) already
            # computed -- otherwise each AV head-of-line-blocks the PE for
            # ~3us per group and the pipeline never tightens.
            delayed_av = []  # (av_tile, mt, at_tile, start, stop, ncg)

            def flush_delayed():
                nonlocal prev_av_mm, last_av_mm
                for av_t, fmt, at_t, st, sp, fncg in delayed_av:
                    mmav = nc.tensor.matmul(
                        av_t[:], v_sb[:, fmt * 128:(fmt + 1) * 128], at_t[:],
                        start=st, stop=sp,
                    )
                    prev_av_mm = mmav
                    last_av_mm = mmav
                    if sp:
                        oT = opool.tile([D, NCH], f32, tag="oT")
                        cp = nc.vector.tensor_copy(oT[:], av_t[:])
                        od = nc.gpsimd.dma_start(out=outT[fncg][:], in_=oT[:])
                        tail_insts.extend([cp, od])
                delayed_av.clear()

            for ncg in range(N_NCH):
                nsl = slice(ncg * NCH, (ncg + 1) * NCH)
                av = avpool.tile([D, NCH], f32, tag="av")
                for mt in range(N_MT):
                    mk = mk_tiles[(ncg, mt)]
                    # PE gate: absorbs the mask DMA wait.
                    nc.tensor.ldweights(mk[:, 0:1])
                    uw = []
                    for p in range(2):
                        # PE gate: absorbs the PSUM-slot-release (ACT) wait.
                        g2 = nc.tensor.ldweights(eyef_sb[:, 0:1])
                        if len(tanh_hist) >= 3:
                            add_dep_helper(g2.ins, tanh_hist[-3].ins,
                                           reason="sc release")
                        if len(madds_hist) >= 3 and madds_hist[-3]:
                            # second PE gate: absorbs the DVE (mask-add)
                            # part of the reused slot's release.
                            g3 = nc.tensor.ldweights(eyef_sb[:, 0:1])
                            for ma in madds_hist[-3]:
                                add_dep_helper(g3.ins, ma.ins,
                                               reason="sc release dve")
                        sch = pspool.tile([128, 2 * NCH], f32, tag="sc",
                                          name=f"sc{p}")
                        madds = []
                        if p == 0:
                            # inject mask^T into each head's bank with a
                            # FULL-ARRAY identity matmul (tiled injects
                            # racing the tiled score accumulation on the
                            # same PSUM addresses hard-fault the PE).
                            for hh in range(2):
                                off = NCH * hh
                                mm = nc.tensor.matmul(
                                    sch[:, off:off + NCH], eyef_sb[:], mk[:],
                                    start=True, stop=False,
                                    skip_group_check=True,
                                )
                                add_dep_helper(mm.ins, g2.ins,
                                               reason="gate order")
                            # packed scores for this pair's two heads
                            for j in range(4):
                                qs = slice(32 * j, 32 * (j + 1))
                                moff = mt * 128 + 32 * j
                                for hh in range(2):
                                    h = 2 * p + hh
                                    hs = slice(32 * h, 32 * (h + 1))
                                    off = NCH * hh
                                    nc.tensor.matmul(
                                        sch[qs, off:off + NCH],
                                        kT_sb[hs, moff:moff + 32],
                                        qT_sb[hs, nsl],
                                        start=False, stop=(j == 3),
                                        tile_position=(32 * h, 32 * j),
                                        skip_group_check=True,
                                    )
                        else:
                            # VectorE carries this pair's mask add: each
                            # bank's scores are ONE K=32/M=128 matmul with
                            # start=True (single writer per bank -- no
                            # packed clears to race), then DVE adds mask^T
                            # into PSUM in place.
                            score_stop = []
                            for hh in range(2):
                                h = 2 * p + hh
                                hs = slice(32 * h, 32 * (h + 1))
                                off = NCH * hh
                                smm = nc.tensor.matmul(
                                    sch[:, off:off + NCH],
                                    kT_sb[hs, mt * 128:(mt + 1) * 128],
                                    qT_sb[hs, nsl],
                                    start=True, stop=True,
                                    tile_position=(32 * h, 0),
                                    skip_group_check=True,
                                )
                                add_dep_helper(smm.ins, g2.ins,
                                               reason="gate order")
                                score_stop.append(smm)
                            # DVE gate: absorbs the mask-DMA wait so the
                            # in-place adds carry only their PE wait.
                            gdmk = gsbpool.tile([1, 8], bf16, tag="gdmk",
                                                name=f"gdmk_{half}", bufs=70)
                            nc.vector.tensor_copy(gdmk[0:1, 0:1],
                                                  mk[0:1, 0:1])
                            for hh in range(2):
                                off = NCH * hh
                                ma = nc.vector.tensor_add(
                                    sch[:, off:off + NCH],
                                    sch[:, off:off + NCH], mk[:])
                                madds.append(ma)
                        madds_hist.append(madds)
                        # spaced ACT gates: advance ACT's observed DVE clock
                        # (th slot releases) and ACT self clock (th WAW) so
                        # the real tanhs carry only their PE wait.
                        if half % GATE_EVERY == 2 and prev_at is not None:
                            gact = gsbpool.tile([1, 8], bf16, tag="gact",
                                                name=f"gact_{half}", bufs=70)
                            nc.scalar.copy(gact[0:1, 0:1], prev_at[0:1, 0:1])
                            gact2 = gsbpool.tile([1, 8], bf16, tag="gact2",
                                                 name=f"gact2_{half}", bufs=70)
                            ga2 = nc.scalar.copy(gact2[0:1, 0:1],
                                                 actsrc[0:1, 0:1])
                            add_dep_helper(ga2.ins, tanh_hist[-1].ins,
                                           reason="th waw")
                        if p == 1:
                            # ACT gate: absorbs the PE score wait so this
                            # tanh carries only the DVE mask-add wait.
                            gactp = gsbpool.tile([1, 8], bf16, tag="gactp",
                                                 name=f"gactp_{half}",
                                                 bufs=70)
                            gp = nc.scalar.copy(gactp[0:1, 0:1],
                                                qT_sb[0:1, 0:1])
                            for smm in score_stop:
                                add_dep_helper(gp.ins, smm.ins,
                                               reason="score order")
                        tht = thpool.tile([128, 2 * NCH], bf16, tag="th",
                                          name=f"th{p}")
                        act = nc.scalar.activation(tht[:], sch[:], TANH)
                        tanh_hist.append(act)
                        # pair-sum on DVE (bf16 SBUF 2x mode)
                        uwt = atpool.tile([128, NCH], bf16,
                                          tag=("u" if p == 0 else "w"))
                        nc.vector.tensor_add(uwt[:], tht[:, 0:NCH],
                                             tht[:, NCH:2 * NCH])
                        uw.append(uwt)
                        half += 1
                        if p == 0:
                            flush_delayed()
                    # DVE gate: absorbs the at-slot-release (PE) wait so the
                    # adds carry at most one sync wait.
                    if prev_av_mm is not None and mt % 2 == 0:
                        gdve = gsbpool.tile([1, 8], bf16, tag="gdve",
                                            name=f"gdve_{ncg}_{mt}", bufs=70)
                        gd = nc.vector.tensor_copy(gdve[0:1, 0:1],
                                                   actsrc[0:1, 0:1])
                        add_dep_helper(gd.ins, prev_av_mm.ins,
                                       reason="at release")
                    at = atpool.tile([128, NCH], bf16, tag="at")
                    nc.vector.tensor_add(at[:], uw[0][:], uw[1][:])
                    prev_at = at
                    # out^T[d, nch] += V'[mt]^T @ A^T[mt] (deferred)
                    delayed_av.append((av, mt, at,
                                       mt == 0, mt == N_MT - 1, ncg))
            flush_delayed()

            tail_insts.extend(tanh_hist[-2:])
            if last_av_mm is not None:
                tail_insts.append(last_av_mm)
            for ti in tail_insts:
                nz = nc.sync.nop(nofuse=True, hint="predrain")
                add_dep_helper(nz.ins, ti.ins, reason="predrain absorb")

    return nc


def get_nc():
    if "nc" not in _NC_CACHE:
        _NC_CACHE["nc"] = _build_nc()
    return _NC_CACHE["nc"]


def _prep_in_maps(x, cond, attention_mask, Wq, bq, Wk, bk, Wv, bv):
    import ml_dtypes

    bf16 = ml_dtypes.bfloat16
    s = 1.0 / math.sqrt(128.0)

    Wq_s = (np.asarray(Wq, np.float32) * s).astype(bf16)
    Wk_b = np.asarray(Wk, np.float32).astype(bf16)
    Wv4 = (np.asarray(Wv, np.float32) * 0.25).astype(bf16)

    def _row0(vec):
        m = np.zeros((D, D), np.float32)
        m[0, :] = vec
        return m.astype(bf16)

    bq_s = _row0(np.asarray(bq, np.float32) * s)
    bk_b = _row0(np.asarray(bk, np.float32))
    bv4 = _row0(np.asarray(bv, np.float32) * 0.25)
    onesm = np.zeros((D, NCH), np.float32)
    onesm[0, :] = 1.0
    onesm = onesm.astype(bf16)
    eyef = np.eye(D, dtype=np.float32).astype(bf16)

    x = np.asarray(x, np.float32)
    cond = np.asarray(cond, np.float32)
    attention_mask = np.asarray(attention_mask, np.float32)

    in_maps = []
    for i in range(B):
        in_maps.append({
            "xT": np.ascontiguousarray(x[i].T).astype(bf16),
            "condT": np.ascontiguousarray(cond[i].T).astype(bf16),
            "maskT": np.ascontiguousarray(attention_mask[i].T).astype(bf16),
            "Wq": Wq_s, "Wk": Wk_b, "Wv4": Wv4,
            "bq": bq_s, "bk": bk_b, "bv4": bv4,
            "onesm": onesm, "eyef": eyef,
        })
    return in_maps


def run(x, cond, flags, attention_mask, Wq, bq, Wk, bk, Wv, bv,
        trace=False, tmpdir=None):
    """Returns (out [B,N,D] float32, exec_time_ns or None)."""
    from concourse.bass_utils import run_bass_kernel_spmd

    nc = get_nc()
    in_maps = _prep_in_maps(x, cond, attention_mask, Wq, bq, Wk, bk, Wv, bv)
    res = run_bass_kernel_spmd(
        nc, in_maps, core_ids=list(range(B)), trace=trace, tmpdir=tmpdir,
    )
    out = np.stack(
        [np.concatenate([np.asarray(r[f"outT{i}"], np.float32)
                         for i in range(N_NCH)], axis=1).T
         for r in res.results], axis=0
    )
    return out, res.exec_time_ns


def kernel(**inputs):
    out, _ = run(**inputs)
    return out
